# revision 1
# baseline (speedup 1.0000x reference)
"""APPNP v2: single interleaved pass per step.

Changes vs baseline:
  - self-loop edges folded into the edge list (u-space weight 1) ->
    no per-tile self-loop add, no accA buffer, unew is write-only per step.
  - one pass over tiles per step: each tile's psum accumulates its A-table
    groups then B-table groups; epilogue is a single scalar_tensor_tensor.
  - all gathers for a step issued upfront, A/B zip-interleaved, matching
    the consumption order of the tile loop.
"""
import sys
sys.path.insert(0, '/opt/trn_rl_repo')
sys.path.insert(0, '/opt/pypackages')

import numpy as np
import ml_dtypes
import concourse.bass as bass
import concourse.bacc as bacc
import concourse.tile as tile
import concourse.mybir as mybir

F32 = mybir.dt.float32
I16 = mybir.dt.int16
BF16 = mybir.dt.bfloat16
AL = mybir.AluOpType

EPS = 1e-5


class Cfg:
    def __init__(self, N=50000, F_IN=512, H=128, K=10, ALPHA=0.1, NC=8, T=49,
                 gdt="bf16", NQ=4, SCRATCH=32768, CHUNK_G=8, REPS=1,
                 MSB=24, OHB=4, PSB=8, ablate=None, SP=True, SELF=False):
        self.ablate = ablate  # None | "gather_only" | "compute_only"
        self.SP = SP
        self.SELF = SELF  # fold self loops into edge list (vs DVE add)
        self.N, self.F_IN, self.H, self.K, self.ALPHA, self.NC, self.T = \
            N, F_IN, H, K, ALPHA, NC, T
        self.MSB, self.OHB, self.PSB = MSB, OHB, PSB
        self.S = T * 128
        self.Npad = NC * self.S
        assert self.Npad >= N
        self.TA = T // 2
        self.TB = T - self.TA
        self.SA = self.TA * 128
        self.SB = self.TB * 128
        self.NRA = NC * self.SA
        self.NRB = NC * self.SB
        assert self.NRA <= 32767 and self.NRB <= 32767
        self.gdt = BF16 if gdt == "bf16" else F32
        self.np_gdt = ml_dtypes.bfloat16 if gdt == "bf16" else np.float32
        self.NQ = NQ
        self.SCRATCH = SCRATCH
        self.CHUNK_G = CHUNK_G
        self.KB = F_IN // 128
        self.REPS = REPS


def preprocess(cfg, x, edge_index, W1, b1, gamma1, beta1, mean1, var1,
               W2, b2, gamma2, beta2, mean2, var2):
    N, S, T, NC = cfg.N, cfg.S, cfg.T, cfg.NC
    SA, SB = cfg.SA, cfg.SB
    src = np.asarray(edge_index[0], dtype=np.int64)
    dst = np.asarray(edge_index[1], dtype=np.int64)

    deg = np.bincount(dst, minlength=N).astype(np.float64) + 1.0
    dinv = 1.0 / np.sqrt(deg)
    dinv2eff = (1.0 - cfg.ALPHA) * (1.0 / deg)
    sqrtdeg = np.sqrt(deg)

    def nm_table(vals_pad):
        out = []
        for c in range(NC):
            v = vals_pad[c * S:(c + 1) * S].reshape(T, 128).T.copy()
            out.append(np.ascontiguousarray(v, dtype=np.float32))
        return out

    pad = lambda v: np.concatenate([v, np.zeros(cfg.Npad - N, v.dtype)])
    dinv_t = nm_table(pad(dinv))
    d2e_t = nm_table(pad(dinv2eff))
    sqd_t = nm_table(pad(sqrtdeg))

    if cfg.SELF:
        # fold self loops in as ordinary edges (u-space weight 1)
        loop = np.arange(N, dtype=np.int64)
        src = np.concatenate([src, loop])
        dst = np.concatenate([dst, loop])
    order = np.argsort(dst, kind="stable")
    src_s, dst_s = src[order], dst[order]

    tile_edges = [[None] * T for _ in range(NC)]
    bounds = np.searchsorted(dst_s, np.arange(0, cfg.Npad + 1, 128))
    for c in range(NC):
        for t in range(T):
            g = c * T + t
            lo, hi = bounds[g], bounds[g + 1]
            s_, d_ = src_s[lo:hi], dst_s[lo:hi] - (c * S + t * 128)
            sc, sj = s_ // S, s_ % S
            mA = sj < SA
            rowA = (sc * SA + sj)[mA]
            rowB = (sc * SB + (sj - SA))[~mA]
            tile_edges[c][t] = (rowA, d_[mA], rowB, d_[~mA])

    GA = [max(max(-(-len(tile_edges[c][t][0]) // 128) for c in range(NC)), 1)
          for t in range(T)]
    GB = [max(max(-(-len(tile_edges[c][t][2]) // 128) for c in range(NC)), 1)
          for t in range(T)]
    prefA = np.concatenate([[0], np.cumsum(GA)])
    prefB = np.concatenate([[0], np.cumsum(GB)])
    NGA, NGB = int(prefA[-1]), int(prefB[-1])
    CH = cfg.CHUNK_G
    NCA, NCB = -(-NGA // CH), -(-NGB // CH)
    NG = NGA + NGB

    meta = dict(GA=GA, GB=GB, NCA=NCA, NCB=NCB, NG=NG)

    def wrap(stream):
        a = stream.reshape(-1, 16)
        return np.ascontiguousarray(np.tile(a.T, (8, 1)), dtype=np.int16)

    in_maps = []
    a1 = (gamma1 / np.sqrt(var1 + EPS)).astype(np.float32)
    c1 = (beta1 - mean1 * a1).astype(np.float32)
    a2 = (gamma2 / np.sqrt(var2 + EPS)).astype(np.float32)
    W2eff = (np.asarray(W2) * a2[None, :]).astype(np.float32)
    c2eff = ((np.asarray(b2) - mean2) * a2 + beta2).astype(np.float32)
    col = lambda v: np.asarray(v, np.float32).reshape(128, 1)

    x_np = np.asarray(x, np.float32)
    for c in range(NC):
        sA = np.zeros(NCA * CH * 128, np.int16)
        sB = np.zeros(NCB * CH * 128, np.int16)
        dl = np.full((128, NG), -1.0, np.float32)
        for t in range(T):
            eA, dA, eB, dB = tile_edges[c][t]
            a0 = int(prefA[t]) * 128
            sA[a0:a0 + len(eA)] = eA.astype(np.int16)
            b0 = int(prefB[t]) * 128
            sB[b0:b0 + len(eB)] = eB.astype(np.int16)
            g0 = int(prefA[t] + prefB[t])
            dcol = np.full(((GA[t] + GB[t]) * 128,), -1.0, np.float32)
            dcol[:len(dA)] = dA
            dcol[GA[t] * 128:GA[t] * 128 + len(dB)] = dB
            dl[:, g0:g0 + GA[t] + GB[t]] = dcol.reshape(GA[t] + GB[t], 128).T
        xs = np.zeros((S, cfg.F_IN), np.float32)
        lo, hi = c * S, min((c + 1) * S, N)
        if hi > lo:
            xs[:hi - lo] = x_np[lo:hi]
        in_maps.append({
            "x": xs,
            "W1": np.asarray(W1, np.float32),
            "W2eff": W2eff,
            "b1c": col(b1), "a1c": col(a1), "c1c": col(c1), "c2c": col(c2eff),
            "dinv": dinv_t[c], "d2e": d2e_t[c], "sqd": sqd_t[c],
            "idxA": wrap(sA), "idxB": wrap(sB), "dstloc": np.ascontiguousarray(dl),
            "tok": np.zeros((128, 8), np.float32),
        })
    return meta, in_maps


def build(cfg, meta):
    NC, T, S, K = cfg.NC, cfg.T, cfg.S, cfg.K
    TA = cfg.TA
    GA, GB, NCA, NCB, NG = (meta[k] for k in ("GA", "GB", "NCA", "NCB", "NG"))
    CH = cfg.CHUNK_G
    gdt = cfg.gdt
    GMAX = max(GA[t] + GB[t] for t in range(T))

    nc = bacc.Bacc("TRN2", target_bir_lowering=False, debug=False,
                   num_devices=NC, num_swdge_queues=cfg.NQ,
                   dynamic_dma_scratch_size=cfg.SCRATCH)

    dram = lambda n, s, d: nc.dram_tensor(n, s, d, kind="ExternalInput")
    x_t = dram("x", [S, cfg.F_IN], F32)
    W1_t = dram("W1", [cfg.F_IN, 128], F32)
    W2_t = dram("W2eff", [128, 128], F32)
    b1_t = dram("b1c", [128, 1], F32)
    a1_t = dram("a1c", [128, 1], F32)
    c1_t = dram("c1c", [128, 1], F32)
    c2_t = dram("c2c", [128, 1], F32)
    dinv_t = dram("dinv", [128, T], F32)
    d2e_t = dram("d2e", [128, T], F32)
    sqd_t = dram("sqd", [128, T], F32)
    idxA_t = dram("idxA", [128, NCA * CH * 8], I16)
    idxB_t = dram("idxB", [128, NCB * CH * 8], I16)
    dst_t = dram("dstloc", [128, NG], F32)
    tok_t = dram("tok", [128, 8], F32)
    out_t = nc.dram_tensor("out", [S, 128], F32, kind="ExternalOutput")
    toko_t = nc.dram_tensor("tok_out", [128, 8], F32, kind="ExternalOutput")

    ag_in_a = [nc.dram_tensor(f"ag_ina{i}", [cfg.SA, 128], gdt) for i in range(2)]
    ag_in_b = [nc.dram_tensor(f"ag_inb{i}", [cfg.SB, 128], gdt) for i in range(2)]
    ag_buf_a = [nc.dram_tensor(f"ag_bufa{i}", [cfg.NRA, 128], gdt,
                               addr_space="Shared") for i in range(2)]
    ag_buf_b = [nc.dram_tensor(f"ag_bufb{i}", [cfg.NRB, 128], gdt,
                               addr_space="Shared") for i in range(2)]

    with tile.TileContext(nc) as tc:
        with (
            tc.tile_pool(name="persist", bufs=1) as P,
        ):
            def load(name, t_, shape, dt_):
                sb = P.tile(shape, dt_, tag=name)
                nc.sync.dma_start(sb[:], t_[:])
                return sb
            W1_sb = load("W1", W1_t.ap().rearrange("(k p) h -> p k h", p=128),
                         [128, cfg.KB, 128], F32)
            W2_sb = load("W2", W2_t, [128, 128], F32)
            b1_sb = load("b1", b1_t, [128, 1], F32)
            a1_sb = load("a1", a1_t, [128, 1], F32)
            c1_sb = load("c1", c1_t, [128, 1], F32)
            c2_sb = load("c2", c2_t, [128, 1], F32)
            dinv_sb = load("dinv", dinv_t, [128, T], F32)
            d2e_sb = load("d2e", d2e_t, [128, T], F32)
            sqd_sb = load("sqd", sqd_t, [128, T], F32)
            idxA_sb = load("idxA", idxA_t, [128, NCA * CH * 8], I16)
            idxB_sb = load("idxB", idxB_t, [128, NCB * CH * 8], I16)
            dst_sb = load("dstloc", dst_t, [128, NG], F32)
            tok_sb = load("tok", tok_t, [128, 8], F32)
            nc.sync.dma_start(toko_t[:], tok_sb[:])

            iota_sb = P.tile([128, 128], F32, tag="iota")
            nc.gpsimd.iota(iota_sb[:], pattern=[[1, 128]], base=0,
                           channel_multiplier=0,
                           allow_small_or_imprecise_dtypes=True)
            iop_sb = P.tile([128, 1], F32, tag="iop")
            nc.gpsimd.iota(iop_sb[:], pattern=[[1, 1]], base=0,
                           channel_multiplier=1,
                           allow_small_or_imprecise_dtypes=True)
            ident = P.tile([128, 128], F32, tag="ident")
            nc.vector.tensor_scalar(ident[:], iota_sb[:], iop_sb[:, 0:1], None,
                                    op0=AL.is_equal)

            au0_sb = P.tile([128, T, 128], F32, tag="au0")
            unew_sb = P.tile([128, T, 128], gdt, tag="unew")
            out_sb = P.tile([128, T, 128], F32, tag="outsb")

            prefA = [0]
            prefB = [0]
            for t in range(T):
                prefA.append(prefA[-1] + GA[t])
                prefB.append(prefB[-1] + GB[t])

            for rep in range(cfg.REPS):
                # --- MLP -> u0 ------------------------------------------------
                with (
                    tc.tile_pool(name="mlp_pT", bufs=2, space="PSUM") as MPT,
                    tc.tile_pool(name="mlp_ps", bufs=1, space="PSUM") as MPS,
                    tc.tile_pool(name="mlp_sb", bufs=3) as MSB_,
                ):
                    for t0 in range(0, T, 4):
                        nt = min(4, T - t0)
                        W = nt * 128
                        xT = MSB_.tile([128, cfg.KB, W], F32, tag="xT")
                        for i in range(nt):
                            xt = MSB_.tile([128, cfg.F_IN], F32, tag="xt")
                            nc.sync.dma_start(
                                xt[:], x_t[(t0 + i) * 128:(t0 + i + 1) * 128, :])
                            for kb in range(cfg.KB):
                                pT = MPT.tile([128, 128], F32, tag="pT")
                                nc.tensor.transpose(
                                    pT[:], xt[:, kb * 128:(kb + 1) * 128], ident[:])
                                nc.scalar.activation(
                                    xT[:, kb, i * 128:(i + 1) * 128], pT[:],
                                    mybir.ActivationFunctionType.Copy)
                        h1p = MPS.tile([128, W], F32, tag="h1p")
                        for kb in range(cfg.KB):
                            nc.tensor.matmul(h1p[:], W1_sb[:, kb, :], xT[:, kb, :],
                                             start=(kb == 0), stop=(kb == cfg.KB - 1))
                        h1 = MSB_.tile([128, W], F32, tag="h1")
                        nc.scalar.activation(h1[:], h1p[:],
                                             mybir.ActivationFunctionType.Relu,
                                             bias=b1_sb[:, 0:1])
                        nc.vector.tensor_scalar(h1[:], h1[:], a1_sb[:, 0:1],
                                                c1_sb[:, 0:1], op0=AL.mult,
                                                op1=AL.add)
                        h2p = MPS.tile([128, W], F32, tag="h2p")
                        nc.tensor.matmul(h2p[:], W2_sb[:], h1[:], start=True,
                                         stop=True)
                        u0T = MSB_.tile([128, W], F32, tag="u0T")
                        nc.vector.tensor_scalar(u0T[:], h2p[:], c2_sb[:, 0:1],
                                                None, op0=AL.add)
                        for i in range(nt):
                            t = t0 + i
                            pT = MPT.tile([128, 128], F32, tag="pT")
                            nc.tensor.transpose(
                                pT[:], u0T[:, i * 128:(i + 1) * 128], ident[:])
                            nc.vector.tensor_scalar(
                                au0_sb[:, t, :], pT[:], dinv_sb[:, t:t + 1],
                                float(cfg.ALPHA), op0=AL.mult, op1=AL.mult)
                            nc.vector.tensor_scalar(
                                unew_sb[:, t, :], pT[:], dinv_sb[:, t:t + 1],
                                None, op0=AL.mult)

                # --- propagation steps ---------------------------------------
                with (
                    tc.tile_pool(name="msgsA", bufs=cfg.MSB // 2) as MSA,
                    tc.tile_pool(name="msgsB", bufs=cfg.MSB // 2) as MSB2,
                    tc.tile_pool(name="oh", bufs=cfg.OHB) as OH,
                    tc.tile_pool(name="psum_t", bufs=cfg.PSB, space="PSUM") as PSA,
                ):
                    MS = MSA
                    def exchange_a(b):
                        nc.sync.dma_start(
                            ag_in_a[b].ap().rearrange("(t p) f -> p t f", p=128),
                            unew_sb[:, 0:TA, :])
                        if cfg.ablate == "no_coll":
                            nc.sync.dma_start(ag_buf_a[b][0:cfg.SA, :],
                                              ag_in_a[b][:])
                        else:
                            nc.gpsimd.collective_compute(
                                "AllGather", AL.bypass,
                                replica_groups=[list(range(NC))],
                                ins=[ag_in_a[b][:]], outs=[ag_buf_a[b][:]],
                            )

                    def exchange_b(b):
                        nc.sync.dma_start(
                            ag_in_b[b].ap().rearrange("(t p) f -> p t f", p=128),
                            unew_sb[:, TA:T, :])
                        if cfg.ablate == "no_coll":
                            nc.sync.dma_start(ag_buf_b[b][0:cfg.SB, :],
                                              ag_in_b[b][:])
                        else:
                            nc.gpsimd.collective_compute(
                                "AllGather", AL.bypass,
                                replica_groups=[list(range(NC))],
                                ins=[ag_in_b[b][:]], outs=[ag_buf_b[b][:]],
                            )
                    exchange_a(0)
                    exchange_b(0)
                    fixedA = fixedB = None
                    if cfg.ablate == "compute_only":
                        # gather 4+4 tiles once; every matmul reads these
                        fixedA, fixedB = [], []
                        for ci in range(4):
                            m = MS.tile([128, CH, 128], gdt, tag="m")
                            nc.gpsimd.dma_gather(
                                m[:], ag_buf_a[0][:, :],
                                idxA_sb[:, ci * CH * 8:(ci + 1) * CH * 8],
                                CH * 128, CH * 128, 128,
                                elem_step=128, queue_num=0)
                            fixedA.append(m)
                            m = MS.tile([128, CH, 128], gdt, tag="m")
                            nc.gpsimd.dma_gather(
                                m[:], ag_buf_b[0][:, :],
                                idxB_sb[:, ci * CH * 8:(ci + 1) * CH * 8],
                                CH * 128, CH * 128, 128,
                                elem_step=128, queue_num=1 % cfg.NQ)
                            fixedB.append(m)
                    for k in range(1, K + 1):
                        tabA = ag_buf_a[(k - 1) % 2][:, :]
                        tabB = ag_buf_b[(k - 1) % 2][:, :]

                        # issue all gathers, zip order matching consumption
                        chunksA = [None] * NCA
                        chunksB = [None] * NCB
                        if cfg.ablate == "compute_only":
                            chunksA = [fixedA[i % 4] for i in range(NCA)]
                            chunksB = [fixedB[i % 4] for i in range(NCB)]
                        else:
                            # A-chunks first on all queues (table A arrives
                            # mid-prior-step), then B-chunks (table B arrives
                            # at prior step end) — B stalls never block A.
                            for ci in range(NCA):
                                m = MSA.tile([128, CH, 128], gdt, tag="mA")
                                nc.gpsimd.dma_gather(
                                    m[:], tabA,
                                    idxA_sb[:, ci * CH * 8:(ci + 1) * CH * 8],
                                    CH * 128, CH * 128, 128,
                                    elem_step=128, queue_num=ci % cfg.NQ,
                                    single_packet=cfg.SP)
                                chunksA[ci] = m
                            for ci in range(NCB):
                                m = MSB2.tile([128, CH, 128], gdt, tag="mB")
                                nc.gpsimd.dma_gather(
                                    m[:], tabB,
                                    idxB_sb[:, ci * CH * 8:(ci + 1) * CH * 8],
                                    CH * 128, CH * 128, 128,
                                    elem_step=128, queue_num=ci % cfg.NQ,
                                    single_packet=cfg.SP)
                                chunksB[ci] = m
                        if cfg.ablate == "gather_only":
                            continue

                        for t in range(T):
                            g0 = prefA[t] + prefB[t]
                            Gt = GA[t] + GB[t]
                            oh = OH.tile([128, GMAX * 128], gdt, tag="oh")
                            nc.vector.tensor_tensor(
                                out=oh[:, 0:Gt * 128].rearrange(
                                    "p (g f) -> p g f", f=128),
                                in0=iota_sb[:].unsqueeze(1).broadcast_to(
                                    [128, Gt, 128]),
                                in1=dst_sb[:, g0:g0 + Gt]
                                    .unsqueeze(2).broadcast_to([128, Gt, 128]),
                                op=AL.is_equal)
                            ps = PSA.tile([128, 128], F32, tag="ps")
                            nj = Gt
                            for j in range(GA[t]):
                                a = prefA[t] + j
                                nc.tensor.matmul(
                                    ps[:], oh[:, j * 128:(j + 1) * 128],
                                    chunksA[a // CH][:, a % CH, :],
                                    start=(j == 0), stop=(nj == GA[t]
                                                          and j == nj - 1))
                            for j in range(GB[t]):
                                b = prefB[t] + j
                                jo = GA[t] + j
                                nc.tensor.matmul(
                                    ps[:], oh[:, jo * 128:(jo + 1) * 128],
                                    chunksB[b // CH][:, b % CH, :],
                                    start=False, stop=(j == GB[t] - 1))
                            if cfg.SELF:
                                tot = ps[:]
                            else:
                                tt = OH.tile([128, 128], F32, tag="tot")
                                nc.vector.tensor_tensor(
                                    out=tt[:], in0=ps[:],
                                    in1=unew_sb[:, t, :], op=AL.add)
                                tot = tt[:]
                            if k < K:
                                nc.vector.scalar_tensor_tensor(
                                    out=unew_sb[:, t, :], in0=tot,
                                    scalar=d2e_sb[:, t:t + 1],
                                    in1=au0_sb[:, t, :],
                                    op0=AL.mult, op1=AL.add)
                                if t == TA - 1:
                                    exchange_a(k % 2)
                            else:
                                fin = OH.tile([128, 128], F32, tag="fin")
                                nc.vector.scalar_tensor_tensor(
                                    out=fin[:], in0=tot,
                                    scalar=d2e_sb[:, t:t + 1],
                                    in1=au0_sb[:, t, :],
                                    op0=AL.mult, op1=AL.add)
                                nc.vector.tensor_scalar(
                                    out_sb[:, t, :], fin[:],
                                    sqd_sb[:, t:t + 1], None, op0=AL.mult)
                        if k < K:
                            exchange_b(k % 2)
                    if cfg.ablate == "gather_only":
                        nc.vector.tensor_scalar(
                            out_sb[:].rearrange("p t f -> p (t f)"),
                            au0_sb[:].rearrange("p t f -> p (t f)"),
                            1.0, None, op0=AL.mult)
            nc.sync.dma_start(out_t.ap().rearrange("(t p) f -> p t f", p=128),
                              out_sb[:])
    nc.compile()
    return nc


def assemble_output(cfg, results):
    full = np.concatenate([results[c]["out"] for c in range(cfg.NC)], axis=0)
    return full[:cfg.N]


# ----------------------------------------------------------------------------
# Harness entry point: full (unsharded) inputs -> full output.
# ----------------------------------------------------------------------------
from concourse.bass_utils import run_bass_kernel_spmd


def kernel(**inputs):
    import numpy as np
    cfg = Cfg(N=50000, F_IN=512, H=128, K=10, ALPHA=0.1, NC=8, T=49,
              gdt="bf16", NQ=4, SCRATCH=32768, CHUNK_G=8, REPS=1,
              MSB=24, OHB=4, PSB=8, SELF=False)
    meta, in_maps = preprocess(cfg, **{k: np.asarray(v)
                                       for k, v in inputs.items()})
    nc = build(cfg, meta)
    last = None
    for _attempt in range(2):
        try:
            res = run_bass_kernel_spmd(nc, in_maps,
                                       core_ids=list(range(cfg.NC)))
            return assemble_output(cfg, res.results).astype(np.float32)
        except Exception as e:  # rare transient NRT failures: retry once
            last = e
    raise last

